# revision 2
# baseline (speedup 1.0000x reference)
"""Trainium2 Bass kernel for the DLGN kernel-machine problem.

Reference computation (fp32):
    ig = inp @ gating[0]; dg = data @ gating[0]
    K  = sig(B*ig) @ sig(B*dg).T
    for l in 1..3:
        ig = ig @ gating[l]; dg = dg @ gating[l]
        K *= (sig(B*ig) @ sig(B*dg).T) / 512
    out = K @ alphas                      # [n_inp]

Shapes: inp [4096, 512], data [8192, 512], gating [4, 512, 512],
alphas [8192]; out [4096] fp32.

Strategy (8 NeuronCores):
  - 2D shard R=2 x C=4: inp rows in 2 groups of 2048, data rows in 4 groups
    of 2048. core = r*C + c computes y_partial[r-block] over its d-block;
    host sums C partials per i-block. R=2,C=4 minimizes replicated gate-chain
    work (4096/R + 8192/C columns).
  - Gate chains run in f32r (tf32-like) with contraction on partitions,
    activations fed transposed from host; zero on-device transposes.
  - tanh-centered fp8 trick: sig(4x) = (1 + tanh(2x))/2, so with
    t = fp8_e4m3(tanh(2x)) stored by the ACT engine,
        4*K_l = 512 + Ti + Td + t_i . t_d
    exactly (for the quantized t). The big K-product matmuls run in fp8
    DoubleRow mode (2 fp8 weights/PE cell, 2x rate, 256-contraction per
    instruction). Centering at tanh cuts fp8 quantization error ~4x vs
    quantizing sig directly (measured 6.8e-3 vs 2.4e-2 final metric).
  - Row-sums: Ti via t_i8^T @ ones8 DoubleRow matmuls -> PSUM [128,1] slots,
    read directly by the combine's per-partition scalar slot; Td via
    ones8^T @ t_d8 -> [1,512], +512 folded, fed back as a rank-1 f32r
    matmul (ones[1,128] x (Td+512)[1,512]) that PRELOADS each K PSUM
    accumulation group before the two DoubleRow matmuls.
  - Combine on DVE: per (stripe, ic, l): kblk = (kps + Ti) * kblk via
    scalar_tensor_tensor; layer 0 multiplies alphas (pre-scaled on host by
    4^-4 * 512^-3 = 2^-35 exact), layer 3 row-sum-accumulates into y.
  - ACT engine runs Tanh + Copy only (both live in the same activation
    table set, unlike Sigmoid+Copy which thrashed the table ~9x).
    d-side chain copies ride ACT; i-side chain copies ride DVE (idle in
    phase A).
"""

import numpy as np

import concourse.tile as tile
from concourse import bacc, mybir
from concourse.bass_utils import run_bass_kernel_spmd

BETA = 4.0
WIDTH = 512
DEPTH = 4
DIM = 512
N_I = 4096
N_D = 8192
R, C = 2, 4
NI_SH = N_I // R  # 2048
ND_SH = N_D // C  # 2048
D_STRIPE = 512
N_STRIPES = ND_SH // D_STRIPE  # 4
I_CHUNKS = NI_SH // 128  # 16
KC = DIM // 128  # 4 contraction chunks
NB = NI_SH // 512  # 4 column blocks in phase A
C_SCALE = (0.25**DEPTH) / float(WIDTH ** (DEPTH - 1))  # 2^-35, exact

F32 = mybir.dt.float32
F32R = mybir.dt.float32r
FP8 = mybir.dt.float8e4
AFT = mybir.ActivationFunctionType
MULT = mybir.AluOpType.mult
ADD = mybir.AluOpType.add
DRM = mybir.MatmulPerfMode.DoubleRow

_NC = None


def _build(repeat=1):
    nc = bacc.Bacc("TRN2", target_bir_lowering=False, debug=False, num_devices=8)

    inpT_d = nc.dram_tensor("inpT", [DIM, NI_SH], F32R, kind="ExternalInput")
    dataT_d = nc.dram_tensor("dataT", [DIM, ND_SH], F32R, kind="ExternalInput")
    gating_d = nc.dram_tensor("gating", [DEPTH, DIM, DIM], F32R, kind="ExternalInput")
    alphas_d = nc.dram_tensor("alphas_s", [128, ND_SH], F32, kind="ExternalInput")
    ones_d = nc.dram_tensor("ones_w", [1, 128], F32R, kind="ExternalInput")
    y_d = nc.dram_tensor("y", [128, I_CHUNKS], F32, kind="ExternalOutput")

    from contextlib import nullcontext

    with tile.TileContext(nc) as tc:
        with (
            tc.tile_pool(name="w", bufs=1) as wpool,
            tc.tile_pool(name="sigi", bufs=1) as sigi_pool,
            tc.tile_pool(name="yp", bufs=1) as ypool,
            tc.tile_pool(name="cst", bufs=1) as cpool,
            tc.tile_pool(name="gpsum", bufs=3, space="PSUM") as gpsum,
            tc.tile_pool(name="kpsum", bufs=3, space="PSUM") as kpsum,
            tc.tile_pool(name="tip", bufs=1, space="PSUM") as tippool,
            tc.tile_pool(name="tdp", bufs=1, space="PSUM") as tdppool,
            tc.For_i(0, repeat, 1) if repeat > 1 else nullcontext(),
        ):
            W = wpool.tile([128, DEPTH, KC, DIM], F32R)
            for l in range(DEPTH):
                nc.sync.dma_start(
                    W[:, l],
                    gating_d.ap()[l].rearrange("(k p) n -> p k n", p=128),
                )
            ones_w = cpool.tile([1, 128], F32R)
            nc.sync.dma_start(ones_w[:], ones_d.ap())
            ones8_t = cpool.tile([128, 2, 16], FP8)
            nc.gpsimd.memset(ones8_t[:], 1.0)
            ones8 = ones8_t[:, :, 0:1]

            ti8 = sigi_pool.tile([128, DEPTH, KC, NI_SH], FP8)
            y_acc = ypool.tile([128, I_CHUNKS], F32)
            nc.gpsimd.memset(y_acc[:], 0.0)

            # Ti row-sums, PSUM-resident all kernel (read by combine scalar slot)
            tips = tippool.tile([128, DEPTH, I_CHUNKS], F32)

            # ---- Phase A: i-side gate chain + t_i8 + Ti ----
            with tc.tile_pool(name="ig", bufs=2) as igpool:
                prev = igpool.tile([128, KC, NI_SH], F32R, tag="ig")
                inpT_r = inpT_d.ap().rearrange("(k p) n -> p k n", p=128)
                for k in range(KC):
                    nc.sync.dma_start(prev[:, k], inpT_r[:, k])
                for l in range(DEPTH):
                    nxt = (
                        igpool.tile([128, KC, NI_SH], F32R, tag="ig", name=f"ig{l}")
                        if l < DEPTH - 1
                        else None
                    )
                    for m in range(KC):
                        for nb in range(NB):
                            sl = slice(nb * 512, (nb + 1) * 512)
                            ps = gpsum.tile([128, 512], F32, tag="gps")
                            for k in range(KC):
                                nc.tensor.matmul(
                                    ps[:],
                                    W[:, l, k, m * 128 : (m + 1) * 128],
                                    prev[:, k, sl],
                                    start=(k == 0),
                                    stop=(k == KC - 1),
                                )
                            nc.scalar.activation(
                                ti8[:, l, m, sl], ps[:], AFT.Tanh, scale=BETA / 2
                            )
                            if nxt is not None:
                                nc.vector.tensor_copy(nxt[:, m, sl], ps[:])
                    for ic in range(I_CHUNKS):
                        isl = slice(ic * 128, (ic + 1) * 128)
                        for h in range(2):
                            nc.tensor.matmul(
                                tips[:, l, ic : ic + 1],
                                ti8[:, l, 2 * h : 2 * h + 2, isl],
                                ones8,
                                start=(h == 0),
                                stop=(h == 1),
                                perf_mode=DRM,
                            )
                    prev = nxt

            # ---- Phase B: d-stripes ----
            with (
                tc.tile_pool(name="dat", bufs=3) as datpool,
                tc.tile_pool(name="dg", bufs=2) as dgpool,
                tc.tile_pool(name="sigd", bufs=2) as sigd_pool,
                tc.tile_pool(name="alp", bufs=2) as alpool,
                tc.tile_pool(name="augd", bufs=2) as augdpool,
                tc.tile_pool(name="kblk", bufs=3) as kpool,
                tc.tile_pool(name="scr", bufs=2) as scrpool,
            ):
                for s in range(N_STRIPES):
                    ssl = slice(s * D_STRIPE, (s + 1) * D_STRIPE)
                    dat = datpool.tile([128, KC, D_STRIPE], F32R, tag="dat")
                    dat_r = dataT_d.ap()[:, ssl].rearrange("(k p) n -> p k n", p=128)
                    for k in range(KC):
                        nc.sync.dma_start(dat[:, k], dat_r[:, k])
                    alp = alpool.tile([128, D_STRIPE], F32, tag="alp")
                    nc.sync.dma_start(alp[:], alphas_d.ap()[:, ssl])

                    td8 = sigd_pool.tile([128, DEPTH, KC, D_STRIPE], FP8, tag="td8")
                    aug_d = augdpool.tile([1, DEPTH, D_STRIPE], F32R, tag="augd")

                    prevd = dat
                    for l in range(DEPTH):
                        nxtd = (
                            dgpool.tile(
                                [128, KC, D_STRIPE], F32R, tag="dg", name=f"dg{s}_{l}"
                            )
                            if l < DEPTH - 1
                            else None
                        )
                        for m in range(KC):
                            ps = gpsum.tile([128, 512], F32, tag="gps")
                            for k in range(KC):
                                nc.tensor.matmul(
                                    ps[:],
                                    W[:, l, k, m * 128 : (m + 1) * 128],
                                    prevd[:, k, :],
                                    start=(k == 0),
                                    stop=(k == KC - 1),
                                )
                            nc.scalar.activation(
                                td8[:, l, m, :], ps[:], AFT.Tanh, scale=BETA / 2
                            )
                            if nxtd is not None:
                                nc.scalar.activation(nxtd[:, m, :], ps[:], AFT.Copy)
                        tdps = tdppool.tile([1, D_STRIPE], F32, tag="tdps")
                        for h in range(2):
                            nc.tensor.matmul(
                                tdps[:],
                                ones8,
                                td8[:, l, 2 * h : 2 * h + 2, :],
                                start=(h == 0),
                                stop=(h == 1),
                                perf_mode=DRM,
                            )
                        nc.vector.tensor_scalar_add(aug_d[:, l], tdps[:], 512.0)
                        prevd = nxtd

                    # K-product: per (ic, l): rank-1 aug preload + 2 DR matmuls,
                    # then combine on DVE with Ti in the scalar slot.
                    for ic in range(I_CHUNKS):
                        isl = slice(ic * 128, (ic + 1) * 128)
                        kblk = kpool.tile([128, D_STRIPE], F32, tag="kblk")
                        for l in range(DEPTH):
                            kps = kpsum.tile([128, 512], F32, tag="kps")
                            nc.tensor.matmul(
                                kps[:], ones_w[:], aug_d[:, l], start=True, stop=False
                            )
                            for h in range(2):
                                nc.tensor.matmul(
                                    kps[:],
                                    ti8[:, l, 2 * h : 2 * h + 2, isl],
                                    td8[:, l, 2 * h : 2 * h + 2, :],
                                    start=False,
                                    stop=(h == 1),
                                    perf_mode=DRM,
                                )
                            ti_ap = tips[:, l, ic : ic + 1]
                            if l == 0:
                                nc.vector.scalar_tensor_tensor(
                                    kblk[:], kps[:], ti_ap, alp[:], ADD, MULT
                                )
                            elif l < DEPTH - 1:
                                nc.vector.scalar_tensor_tensor(
                                    kblk[:], kps[:], ti_ap, kblk[:], ADD, MULT
                                )
                            else:
                                part = scrpool.tile([128, 1], F32, tag="part")
                                nc.vector.scalar_tensor_tensor(
                                    kblk[:],
                                    kps[:],
                                    ti_ap,
                                    kblk[:],
                                    ADD,
                                    MULT,
                                    accum_out=part[:, 0:1],
                                )
                                nc.vector.tensor_add(
                                    y_acc[:, ic : ic + 1],
                                    y_acc[:, ic : ic + 1],
                                    part[:, 0:1],
                                )

            nc.sync.dma_start(y_d.ap(), y_acc[:])

    nc.compile()
    return nc


def _get_nc():
    global _NC
    if _NC is None:
        _NC = _build()
    return _NC


def make_in_maps(inp, data, gating, alphas):
    inp = np.ascontiguousarray(np.asarray(inp, dtype=np.float32))
    data = np.ascontiguousarray(np.asarray(data, dtype=np.float32))
    gating = np.ascontiguousarray(np.asarray(gating, dtype=np.float32))
    alphas = np.asarray(alphas, dtype=np.float32) * np.float32(C_SCALE)
    ones_w = np.ones((1, 128), np.float32)

    in_maps = []
    for r in range(R):
        inpT = np.ascontiguousarray(inp[r * NI_SH : (r + 1) * NI_SH].T)
        for c in range(C):
            dataT = np.ascontiguousarray(data[c * ND_SH : (c + 1) * ND_SH].T)
            al = np.ascontiguousarray(
                np.broadcast_to(alphas[c * ND_SH : (c + 1) * ND_SH], (128, ND_SH))
            )
            in_maps.append(
                {
                    "inpT": inpT,
                    "dataT": dataT,
                    "gating": gating,
                    "alphas_s": al,
                    "ones_w": ones_w,
                }
            )
    return in_maps


def kernel(inp, data, gating, alphas):
    nc = _get_nc()
    in_maps = make_in_maps(inp, data, gating, alphas)
    res = run_bass_kernel_spmd(nc, in_maps, core_ids=list(range(R * C))).results

    y = np.empty(N_I, dtype=np.float32)
    for r in range(R):
        acc = res[r * C]["y"].T.reshape(NI_SH).copy()
        for c in range(1, C):
            acc += res[r * C + c]["y"].T.reshape(NI_SH)
        y[r * NI_SH : (r + 1) * NI_SH] = acc
    return y


# revision 13
# speedup vs baseline: 1.7245x; 1.7245x over previous
"""Trainium2 Bass kernel for the DLGN kernel-machine problem.

Reference computation (fp32):
    ig = inp @ gating[0]; dg = data @ gating[0]
    K  = sig(B*ig) @ sig(B*dg).T
    for l in 1..3:
        ig = ig @ gating[l]; dg = dg @ gating[l]
        K *= (sig(B*ig) @ sig(B*dg).T) / 512
    out = K @ alphas                      # [n_inp]

Shapes: inp [4096, 512], data [8192, 512], gating [4, 512, 512],
alphas [8192]; out [4096] fp32.

Strategy (8 NeuronCores):
  - 2D shard R=2 x C=4: inp rows in 2 groups of 2048, data rows in 4 groups
    of 2048. core = r*C + c computes y_partial[r-block] over its d-block;
    host sums C partials per i-block. R=2,C=4 minimizes replicated gate-chain
    work (4096/R + 8192/C columns).
  - Gate chains run in bf16 (inputs/weights/intermediates), contraction on
    partitions, activations fed transposed from host; zero on-device
    transposes. bf16 halves DMA traffic and weight-load time vs f32r; its
    rounding adds <1e-3 to the final metric.
  - Asymmetric-centering fp8 trick: with s = sig(4x) and t = tanh(2x)
    (so s_d = (1+t_d)/2), per layer
        2*K_l = Si + s_i8 . t_d8
    where Si = rowsum(s_i8) is EXACT (fp8 matmul vs ones, f32 PSUM accum)
    and rides the per-partition scalar/bias slot of the combine - no PSUM
    preload instructions at all. Centering the d-side routes the bulk of
    K_l through the exact Si path, cutting fp8 quantization error ~2x vs
    naive fp8 (measured 1.25e-2 final metric vs 2.4e-2; gate is 2e-2).
  - The big K-product matmuls run as fp8e4 DoubleRow (2 fp8 weights/PE
    cell, 256-contraction per instruction, ~2x rate): 2 instructions per
    (stripe, i-chunk, layer) with NO dtype mode switches anywhere in the
    hot loop (f32r instructions interleaved with fp8 cost ~750ns each,
    measured; all-fp8 avoids that entirely).
  - Si row-sums: s_i8^T @ ones8 DoubleRow -> PSUM [128,1] slots, copied
    once to SBUF after phase A.
  - Combine, split across three engines per (stripe, ic): layer 0 on DVE
    (stt: (kps+Si)*alphas), layers 1-2 via ACT Identity(kps + Si-bias) ->
    SBUF then Pool multiply (Pool cannot read PSUM), layer 3 on DVE with
    accum_out row-reduce into y. alphas pre-scaled on host by
    (1/2)^4 * 512^-3 = 2^-31 exact.
  - ACT runs Sigmoid/Tanh/Identity/Copy which all live in ONE activation
    table set (sigmoid_and_others) - single table load, no thrash.
"""

import numpy as np

import concourse.tile as tile
from concourse import bacc, mybir
from concourse.bass_utils import run_bass_kernel_spmd

BETA = 4.0
WIDTH = 512
DEPTH = 4
DIM = 512
N_I = 4096
N_D = 8192
R, C = 2, 4
NI_SH = N_I // R  # 2048
ND_SH = N_D // C  # 2048
D_STRIPE = 512
N_STRIPES = ND_SH // D_STRIPE  # 4
I_CHUNKS = NI_SH // 128  # 16
KC = DIM // 128  # 4 contraction chunks
C_SCALE = (0.5**DEPTH) / float(WIDTH ** (DEPTH - 1))  # 2^-31, exact

F32 = mybir.dt.float32
BF16 = mybir.dt.bfloat16
FP8 = mybir.dt.float8e4
AFT = mybir.ActivationFunctionType
MULT = mybir.AluOpType.mult
ADD = mybir.AluOpType.add
DRM = mybir.MatmulPerfMode.DoubleRow

_NC = None


def _gate_layer(nc, gpsum, W, prev, nxt, out8, aft, l, ncols):
    """One gate-chain layer over `ncols` columns: bf16 matmul chain +
    activation to fp8 + (for l<3) chain copy for the next layer."""
    for m in range(KC):
        for nb in range(ncols // 512):
            sl = slice(nb * 512, (nb + 1) * 512)
            ps = gpsum.tile([128, 512], F32, tag="gps")
            for k in range(KC):
                nc.tensor.matmul(
                    ps[:],
                    W[:, l, k, m * 128 : (m + 1) * 128],
                    prev[:, k, sl],
                    start=(k == 0),
                    stop=(k == KC - 1),
                )
            nc.scalar.activation(
                out8[:, l, m, sl],
                ps[:],
                aft,
                scale=BETA if aft == AFT.Sigmoid else BETA / 2,
            )
            if nxt is not None:
                if aft == AFT.Sigmoid:
                    nc.vector.tensor_copy(nxt[:, m, sl], ps[:])
                else:
                    nc.scalar.activation(nxt[:, m, sl], ps[:], AFT.Copy)


def _k_combine(nc, kpsum, kpool, upool, scrpool, ti8, td8, tisb, alp, y_acc, ic):
    """K-product + combine for one (stripe, i-chunk): per layer 2 fp8
    DoubleRow matmuls, then the running product split across engines."""
    isl = slice(ic * 128, (ic + 1) * 128)
    kblk = kpool.tile([128, D_STRIPE], F32, tag="kblk")
    for l in range(DEPTH):
        kps = kpsum.tile([128, 512], F32, tag="kps")
        for h in range(2):
            nc.tensor.matmul(
                kps[:],
                ti8[:, l, 2 * h : 2 * h + 2, isl],
                td8[:, l, 2 * h : 2 * h + 2, :],
                start=(h == 0),
                stop=(h == 1),
                perf_mode=DRM,
            )
        ti_ap = tisb[:, l, ic : ic + 1]
        if l == 0:
            nc.vector.scalar_tensor_tensor(kblk[:], kps[:], ti_ap, alp[:], ADD, MULT)
        elif l < DEPTH - 1:
            u = upool.tile([128, D_STRIPE], F32, tag="u")
            nc.scalar.activation(u[:], kps[:], AFT.Identity, bias=ti_ap)
            nc.gpsimd.tensor_mul(kblk[:], u[:], kblk[:])
        else:
            part = scrpool.tile([128, 1], F32, tag="part")
            nc.vector.scalar_tensor_tensor(
                kblk[:], kps[:], ti_ap, kblk[:], ADD, MULT, accum_out=part[:, 0:1]
            )
            nc.vector.tensor_add(
                y_acc[:, ic : ic + 1], y_acc[:, ic : ic + 1], part[:, 0:1]
            )


def _build(repeat=1):
    nc = bacc.Bacc("TRN2", target_bir_lowering=False, debug=False, num_devices=8)

    inpT_d = nc.dram_tensor("inpT", [DIM, NI_SH], BF16, kind="ExternalInput")
    dataT_d = nc.dram_tensor("dataT", [DIM, ND_SH], BF16, kind="ExternalInput")
    gating_d = nc.dram_tensor("gating", [DEPTH, DIM, DIM], BF16, kind="ExternalInput")
    alphas_d = nc.dram_tensor("alphas_s", [128, ND_SH], F32, kind="ExternalInput")
    y_d = nc.dram_tensor("y", [128, I_CHUNKS], F32, kind="ExternalOutput")

    from contextlib import nullcontext

    with tile.TileContext(nc) as tc:
        with (
            tc.tile_pool(name="w", bufs=1) as wpool,
            tc.tile_pool(name="sigi", bufs=1) as sigi_pool,
            tc.tile_pool(name="yp", bufs=1) as ypool,
            tc.tile_pool(name="cst", bufs=1) as cpool,
            tc.tile_pool(name="gpsum", bufs=3, space="PSUM") as gpsum,
            tc.tile_pool(name="kpsum", bufs=4, space="PSUM") as kpsum,
            tc.tile_pool(name="tip", bufs=1, space="PSUM") as tippool,
            tc.For_i(0, repeat, 1) if repeat > 1 else nullcontext(),
        ):
            W = wpool.tile([128, DEPTH, KC, DIM], BF16)
            for l in range(DEPTH):
                nc.sync.dma_start(
                    W[:, l],
                    gating_d.ap()[l].rearrange("(k p) n -> p k n", p=128),
                )
            ones8_t = cpool.tile([128, 2, 16], FP8)
            nc.gpsimd.memset(ones8_t[:], 1.0)
            ones8 = ones8_t[:, :, 0:1]

            ti8 = sigi_pool.tile([128, DEPTH, KC, NI_SH], FP8)
            y_acc = ypool.tile([128, I_CHUNKS], F32)
            nc.gpsimd.memset(y_acc[:], 0.0)

            # Si row-sums: accumulate in PSUM, then copy once to SBUF
            tips = tippool.tile([128, DEPTH, I_CHUNKS], F32)
            tisb = ypool.tile([128, DEPTH, I_CHUNKS], F32, name="tisb")

            # ---- Phase A: i-side gate chain (sigmoid -> fp8) + Si ----
            with tc.tile_pool(name="ig", bufs=2) as igpool:
                prev = igpool.tile([128, KC, NI_SH], BF16, tag="ig")
                inpT_r = inpT_d.ap().rearrange("(k p) n -> p k n", p=128)
                for k in range(KC):
                    nc.sync.dma_start(prev[:, k], inpT_r[:, k])
                for l in range(DEPTH):
                    nxt = (
                        igpool.tile([128, KC, NI_SH], BF16, tag="ig", name=f"ig{l}")
                        if l < DEPTH - 1
                        else None
                    )
                    _gate_layer(nc, gpsum, W, prev, nxt, ti8, AFT.Sigmoid, l, NI_SH)
                    for ic in range(I_CHUNKS):
                        isl = slice(ic * 128, (ic + 1) * 128)
                        for h in range(2):
                            nc.tensor.matmul(
                                tips[:, l, ic : ic + 1],
                                ti8[:, l, 2 * h : 2 * h + 2, isl],
                                ones8,
                                start=(h == 0),
                                stop=(h == 1),
                                perf_mode=DRM,
                            )
                    prev = nxt

            nc.vector.tensor_copy(tisb[:], tips[:])

            # ---- Phase B: d-stripes (tanh -> fp8) ----
            with (
                tc.tile_pool(name="dat", bufs=3) as datpool,
                tc.tile_pool(name="dg", bufs=2) as dgpool,
                tc.tile_pool(name="sigd", bufs=2) as sigd_pool,
                tc.tile_pool(name="alp", bufs=2) as alpool,
                tc.tile_pool(name="kblk", bufs=3) as kpool,
                tc.tile_pool(name="u", bufs=4) as upool,
                tc.tile_pool(name="scr", bufs=2) as scrpool,
            ):
                for s in range(N_STRIPES):
                    ssl = slice(s * D_STRIPE, (s + 1) * D_STRIPE)
                    dat = datpool.tile([128, KC, D_STRIPE], BF16, tag="dat")
                    dat_r = dataT_d.ap()[:, ssl].rearrange("(k p) n -> p k n", p=128)
                    for k in range(KC):
                        nc.sync.dma_start(dat[:, k], dat_r[:, k])
                    alp = alpool.tile([128, D_STRIPE], F32, tag="alp")
                    nc.sync.dma_start(alp[:], alphas_d.ap()[:, ssl])

                    td8 = sigd_pool.tile([128, DEPTH, KC, D_STRIPE], FP8, tag="td8")

                    prevd = dat
                    for l in range(DEPTH):
                        nxtd = (
                            dgpool.tile(
                                [128, KC, D_STRIPE], BF16, tag="dg", name=f"dg{s}_{l}"
                            )
                            if l < DEPTH - 1
                            else None
                        )
                        _gate_layer(nc, gpsum, W, prevd, nxtd, td8, AFT.Tanh, l, D_STRIPE)
                        prevd = nxtd

                    for ic in range(I_CHUNKS):
                        _k_combine(
                            nc, kpsum, kpool, upool, scrpool,
                            ti8, td8, tisb, alp, y_acc, ic,
                        )

            nc.sync.dma_start(y_d.ap(), y_acc[:])

    nc.compile()
    return nc


def _get_nc():
    global _NC
    if _NC is None:
        _NC = _build()
    return _NC


def make_in_maps(inp, data, gating, alphas):
    import ml_dtypes

    bf = ml_dtypes.bfloat16
    inp = np.ascontiguousarray(np.asarray(inp, dtype=np.float32).astype(bf))
    data = np.ascontiguousarray(np.asarray(data, dtype=np.float32).astype(bf))
    gating = np.ascontiguousarray(np.asarray(gating, dtype=np.float32).astype(bf))
    alphas = np.asarray(alphas, dtype=np.float32) * np.float32(C_SCALE)

    in_maps = []
    for r in range(R):
        inpT = np.ascontiguousarray(inp[r * NI_SH : (r + 1) * NI_SH].T)
        for c in range(C):
            dataT = np.ascontiguousarray(data[c * ND_SH : (c + 1) * ND_SH].T)
            al = np.ascontiguousarray(
                np.broadcast_to(alphas[c * ND_SH : (c + 1) * ND_SH], (128, ND_SH))
            )
            in_maps.append(
                {"inpT": inpT, "dataT": dataT, "gating": gating, "alphas_s": al}
            )
    return in_maps


def kernel(inp, data, gating, alphas):
    nc = _get_nc()
    in_maps = make_in_maps(inp, data, gating, alphas)
    res = run_bass_kernel_spmd(nc, in_maps, core_ids=list(range(R * C))).results

    y = np.empty(N_I, dtype=np.float32)
    for r in range(R):
        acc = res[r * C]["y"].T.reshape(NI_SH).copy()
        for c in range(1, C):
            acc += res[r * C + c]["y"].T.reshape(NI_SH)
        y[r * NI_SH : (r + 1) * NI_SH] = acc
    return y


# revision 14
# speedup vs baseline: 1.7571x; 1.0189x over previous
"""Trainium2 Bass kernel for the DLGN kernel-machine problem.

Reference computation (fp32):
    ig = inp @ gating[0]; dg = data @ gating[0]
    K  = sig(B*ig) @ sig(B*dg).T
    for l in 1..3:
        ig = ig @ gating[l]; dg = dg @ gating[l]
        K *= (sig(B*ig) @ sig(B*dg).T) / 512
    out = K @ alphas                      # [n_inp]

Shapes: inp [4096, 512], data [8192, 512], gating [4, 512, 512],
alphas [8192]; out [4096] fp32.

Strategy (8 NeuronCores):
  - 2D shard R=2 x C=4: inp rows in 2 groups of 2048, data rows in 4 groups
    of 2048. core = r*C + c computes y_partial[r-block] over its d-block;
    host sums C partials per i-block. R=2,C=4 minimizes replicated gate-chain
    work (4096/R + 8192/C columns).
  - Gate chains run in bf16 (inputs/weights/intermediates), contraction on
    partitions, activations fed transposed from host; zero on-device
    transposes. bf16 halves DMA traffic and weight-load time vs f32r; its
    rounding adds <1e-3 to the final metric.
  - Asymmetric-centering fp8 trick: with s = sig(4x) and t = tanh(2x)
    (so s_d = (1+t_d)/2), per layer
        2*K_l = Si + s_i8 . t_d8
    where Si = rowsum(s_i8) is EXACT (fp8 matmul vs ones, f32 PSUM accum)
    and rides the per-partition scalar/bias slot of the combine - no PSUM
    preload instructions at all. Centering the d-side routes the bulk of
    K_l through the exact Si path, cutting fp8 quantization error ~2x vs
    naive fp8 (measured 1.25e-2 final metric vs 2.4e-2; gate is 2e-2).
  - The big K-product matmuls run as fp8e4 DoubleRow (2 fp8 weights/PE
    cell, 256-contraction per instruction, ~2x rate): 2 instructions per
    (stripe, i-chunk, layer) with NO dtype mode switches anywhere in the
    hot loop (f32r instructions interleaved with fp8 cost ~750ns each,
    measured; all-fp8 avoids that entirely).
  - Si row-sums: s_i8^T @ ones8 DoubleRow -> PSUM [128,1] slots, copied
    once to SBUF after phase A.
  - Combine, split across three engines per (stripe, ic): layer 0 on DVE
    (stt: (kps+Si)*alphas), layers 1-2 via ACT Identity(kps + Si-bias) ->
    SBUF then Pool multiply (Pool cannot read PSUM), layer 3 on DVE with
    accum_out row-reduce into y. alphas pre-scaled on host by
    (1/2)^4 * 512^-3 = 2^-31 exact.
  - ACT runs Sigmoid/Tanh/Identity/Copy which all live in ONE activation
    table set (sigmoid_and_others) - single table load, no thrash.
"""

import numpy as np

import concourse.tile as tile
from concourse import bacc, mybir
from concourse.bass_utils import run_bass_kernel_spmd

BETA = 4.0
WIDTH = 512
DEPTH = 4
DIM = 512
N_I = 4096
N_D = 8192
R, C = 2, 4
NI_SH = N_I // R  # 2048
ND_SH = N_D // C  # 2048
D_STRIPE = 512
N_STRIPES = ND_SH // D_STRIPE  # 4
I_CHUNKS = NI_SH // 128  # 16
KC = DIM // 128  # 4 contraction chunks
C_SCALE = (0.5**DEPTH) / float(WIDTH ** (DEPTH - 1))  # 2^-31, exact

F32 = mybir.dt.float32
BF16 = mybir.dt.bfloat16
FP8 = mybir.dt.float8e4
AFT = mybir.ActivationFunctionType
MULT = mybir.AluOpType.mult
ADD = mybir.AluOpType.add
DRM = mybir.MatmulPerfMode.DoubleRow

_NC = None


def _gate_layer(nc, gpsum, W, prev, nxt, out8, aft, l, ncols):
    """One gate-chain layer over `ncols` columns: bf16 matmul chain +
    activation to fp8 + (for l<3) chain copy for the next layer."""
    for m in range(KC):
        for nb in range(ncols // 512):
            sl = slice(nb * 512, (nb + 1) * 512)
            ps = gpsum.tile([128, 512], F32, tag="gps")
            for k in range(KC):
                nc.tensor.matmul(
                    ps[:],
                    W[:, l, k, m * 128 : (m + 1) * 128],
                    prev[:, k, sl],
                    start=(k == 0),
                    stop=(k == KC - 1),
                )
            nc.scalar.activation(
                out8[:, l, m, sl],
                ps[:],
                aft,
                scale=BETA if aft == AFT.Sigmoid else BETA / 2,
            )
            if nxt is not None:
                nc.vector.tensor_copy(nxt[:, m, sl], ps[:])


def _k_combine(nc, kpsum, kpool, upool, scrpool, ti8, td8, tisb, alp, y_acc, ic):
    """K-product + combine for one (stripe, i-chunk): per layer 2 fp8
    DoubleRow matmuls, then the running product split across engines."""
    isl = slice(ic * 128, (ic + 1) * 128)
    kblk = kpool.tile([128, D_STRIPE], F32, tag="kblk")
    for l in range(DEPTH):
        kps = kpsum.tile([128, 512], F32, tag="kps")
        for h in range(2):
            nc.tensor.matmul(
                kps[:],
                ti8[:, l, 2 * h : 2 * h + 2, isl],
                td8[:, l, 2 * h : 2 * h + 2, :],
                start=(h == 0),
                stop=(h == 1),
                perf_mode=DRM,
            )
        ti_ap = tisb[:, l, ic : ic + 1]
        if l == 0:
            nc.vector.scalar_tensor_tensor(kblk[:], kps[:], ti_ap, alp[:], ADD, MULT)
        elif l < DEPTH - 1:
            u = upool.tile([128, D_STRIPE], F32, tag="u")
            nc.scalar.activation(u[:], kps[:], AFT.Identity, bias=ti_ap)
            nc.gpsimd.tensor_mul(kblk[:], u[:], kblk[:])
        else:
            part = scrpool.tile([128, 1], F32, tag="part")
            nc.vector.scalar_tensor_tensor(
                kblk[:], kps[:], ti_ap, kblk[:], ADD, MULT, accum_out=part[:, 0:1]
            )
            nc.vector.tensor_add(
                y_acc[:, ic : ic + 1], y_acc[:, ic : ic + 1], part[:, 0:1]
            )


def _build(repeat=1):
    nc = bacc.Bacc("TRN2", target_bir_lowering=False, debug=False, num_devices=8)

    inpT_d = nc.dram_tensor("inpT", [DIM, NI_SH], BF16, kind="ExternalInput")
    dataT_d = nc.dram_tensor("dataT", [DIM, ND_SH], BF16, kind="ExternalInput")
    gating_d = nc.dram_tensor("gating", [DEPTH, DIM, DIM], BF16, kind="ExternalInput")
    alphas_d = nc.dram_tensor("alphas_s", [128, ND_SH], F32, kind="ExternalInput")
    y_d = nc.dram_tensor("y", [128, I_CHUNKS], F32, kind="ExternalOutput")

    from contextlib import nullcontext

    with tile.TileContext(nc) as tc:
        with (
            tc.tile_pool(name="w", bufs=1) as wpool,
            tc.tile_pool(name="sigi", bufs=1) as sigi_pool,
            tc.tile_pool(name="yp", bufs=1) as ypool,
            tc.tile_pool(name="cst", bufs=1) as cpool,
            tc.tile_pool(name="gpsum", bufs=3, space="PSUM") as gpsum,
            tc.tile_pool(name="kpsum", bufs=4, space="PSUM") as kpsum,
            tc.tile_pool(name="tip", bufs=1, space="PSUM") as tippool,
            tc.For_i(0, repeat, 1) if repeat > 1 else nullcontext(),
        ):
            W = wpool.tile([128, DEPTH, KC, DIM], BF16)
            for l in range(DEPTH):
                nc.sync.dma_start(
                    W[:, l],
                    gating_d.ap()[l].rearrange("(k p) n -> p k n", p=128),
                )
            ones8_t = cpool.tile([128, 2, 16], FP8)
            nc.gpsimd.memset(ones8_t[:], 1.0)
            ones8 = ones8_t[:, :, 0:1]

            ti8 = sigi_pool.tile([128, DEPTH, KC, NI_SH], FP8)
            y_acc = ypool.tile([128, I_CHUNKS], F32)
            nc.gpsimd.memset(y_acc[:], 0.0)

            # Si row-sums: accumulate in PSUM, then copy once to SBUF
            tips = tippool.tile([128, DEPTH, I_CHUNKS], F32)
            tisb = ypool.tile([128, DEPTH, I_CHUNKS], F32, name="tisb")

            # ---- Phase A: i-side gate chain (sigmoid -> fp8) + Si ----
            with tc.tile_pool(name="ig", bufs=2) as igpool:
                prev = igpool.tile([128, KC, NI_SH], BF16, tag="ig")
                inpT_r = inpT_d.ap().rearrange("(k p) n -> p k n", p=128)
                for k in range(KC):
                    nc.sync.dma_start(prev[:, k], inpT_r[:, k])
                for l in range(DEPTH):
                    nxt = (
                        igpool.tile([128, KC, NI_SH], BF16, tag="ig", name=f"ig{l}")
                        if l < DEPTH - 1
                        else None
                    )
                    _gate_layer(nc, gpsum, W, prev, nxt, ti8, AFT.Sigmoid, l, NI_SH)
                    for ic in range(I_CHUNKS):
                        isl = slice(ic * 128, (ic + 1) * 128)
                        for h in range(2):
                            nc.tensor.matmul(
                                tips[:, l, ic : ic + 1],
                                ti8[:, l, 2 * h : 2 * h + 2, isl],
                                ones8,
                                start=(h == 0),
                                stop=(h == 1),
                                perf_mode=DRM,
                            )
                    prev = nxt

            nc.vector.tensor_copy(tisb[:], tips[:])

            # ---- Phase B: d-stripes (tanh -> fp8) ----
            with (
                tc.tile_pool(name="dat", bufs=3) as datpool,
                tc.tile_pool(name="dg", bufs=3) as dgpool,
                tc.tile_pool(name="sigd", bufs=2) as sigd_pool,
                tc.tile_pool(name="alp", bufs=2) as alpool,
                tc.tile_pool(name="kblk", bufs=4) as kpool,
                tc.tile_pool(name="u", bufs=6) as upool,
                tc.tile_pool(name="scr", bufs=4) as scrpool,
            ):
                for s in range(N_STRIPES):
                    ssl = slice(s * D_STRIPE, (s + 1) * D_STRIPE)
                    dat = datpool.tile([128, KC, D_STRIPE], BF16, tag="dat")
                    dat_r = dataT_d.ap()[:, ssl].rearrange("(k p) n -> p k n", p=128)
                    for k in range(KC):
                        nc.sync.dma_start(dat[:, k], dat_r[:, k])
                    alp = alpool.tile([128, D_STRIPE], F32, tag="alp")
                    nc.sync.dma_start(alp[:], alphas_d.ap()[:, ssl])

                    td8 = sigd_pool.tile([128, DEPTH, KC, D_STRIPE], FP8, tag="td8")

                    prevd = dat
                    for l in range(DEPTH):
                        nxtd = (
                            dgpool.tile(
                                [128, KC, D_STRIPE], BF16, tag="dg", name=f"dg{s}_{l}"
                            )
                            if l < DEPTH - 1
                            else None
                        )
                        _gate_layer(nc, gpsum, W, prevd, nxtd, td8, AFT.Tanh, l, D_STRIPE)
                        prevd = nxtd

                    for ic in range(I_CHUNKS):
                        _k_combine(
                            nc, kpsum, kpool, upool, scrpool,
                            ti8, td8, tisb, alp, y_acc, ic,
                        )

            nc.sync.dma_start(y_d.ap(), y_acc[:])

    nc.compile()
    return nc


def _get_nc():
    global _NC
    if _NC is None:
        _NC = _build()
    return _NC


def make_in_maps(inp, data, gating, alphas):
    import ml_dtypes

    bf = ml_dtypes.bfloat16
    inp = np.ascontiguousarray(np.asarray(inp, dtype=np.float32).astype(bf))
    data = np.ascontiguousarray(np.asarray(data, dtype=np.float32).astype(bf))
    gating = np.ascontiguousarray(np.asarray(gating, dtype=np.float32).astype(bf))
    alphas = np.asarray(alphas, dtype=np.float32) * np.float32(C_SCALE)

    in_maps = []
    for r in range(R):
        inpT = np.ascontiguousarray(inp[r * NI_SH : (r + 1) * NI_SH].T)
        for c in range(C):
            dataT = np.ascontiguousarray(data[c * ND_SH : (c + 1) * ND_SH].T)
            al = np.ascontiguousarray(
                np.broadcast_to(alphas[c * ND_SH : (c + 1) * ND_SH], (128, ND_SH))
            )
            in_maps.append(
                {"inpT": inpT, "dataT": dataT, "gating": gating, "alphas_s": al}
            )
    return in_maps


def kernel(inp, data, gating, alphas):
    nc = _get_nc()
    in_maps = make_in_maps(inp, data, gating, alphas)
    res = run_bass_kernel_spmd(nc, in_maps, core_ids=list(range(R * C))).results

    y = np.empty(N_I, dtype=np.float32)
    for r in range(R):
        acc = res[r * C]["y"].T.reshape(NI_SH).copy()
        for c in range(1, C):
            acc += res[r * C + c]["y"].T.reshape(NI_SH)
        y[r * NI_SH : (r + 1) * NI_SH] = acc
    return y
